# revision 27
# baseline (speedup 1.0000x reference)
"""3x3 valid cross-correlation of 64 1024x1024 f32 images on 8 TRN2 NeuronCores.

Pure data parallel over batch (8 images/core). The conv is memory-bound, so
everything is about HBM traffic and DMA efficiency:

- fp16 I/O: inputs f32->fp16 and outputs fp16->f32 on the host; all device
  DMA moves 2-byte elements (L2 rel err ~3.4e-4, gate is 2e-2).
- Compute: each image is 8 row-blocks (128 input rows -> 126 output rows,
  2-row overlap). TensorEngine does 3 PSUM-accumulated fp16 matmuls per
  512-wide column segment: a banded [128, 126] stationary applies the 3
  vertical taps of kernel column dj; the moving operand is the image block
  column-shifted by dj (free-dim AP offset). The last 14 output rows of ALL
  8 images run in one shared "tail" pass (8x16 input rows packed on 128
  partitions), replacing 8 near-empty per-image tail passes.
- Stores: TRN2 HBM write bandwidth collapses ~5x on small or misaligned
  lines (4088B lines ~82 GB/s vs 32KB lines ~370 GB/s), so outputs are
  written block-major (partition p = rows {p, 126+p, ...} contiguous) and
  unscrambled on the host. The 6 middle images are stored in pairs for
  32KB/partition lines; images 0/7 go as singles to keep fill/drain short.
- Loads ride the SP HWDGE ring, stores the ACT ring; image 0's load is
  split in two so the TensorEngine starts ~4us earlier.
"""

import numpy as np

import bass_rust
import concourse.bacc as bacc
import concourse.mybir as mybir
from concourse.tile import TileContext

B = 64          # batch
D = 1024        # image side
O = D - 2       # 1022 output side
N_CORES = 8
BPC = B // N_CORES  # images per core
BLK = 126       # output rows per full block
NBLK = 8        # full blocks per image; tail handled separately
TAIL_M = O - NBLK * BLK   # 14 tail output rows per image
TAIL_K = 16     # tail input rows per image (1008..1023)

_F32 = mybir.dt.float32
_F16 = mybir.dt.float16


def _make_bands(ker):
    """Banded stationary matrices from the 3x3 kernel (fp16).

    A[k, dj, m] = ker[k-m, dj]  (k-m in 0..2) -> 126 output rows per block
    T8[i*16+q+di, dj, i*14+q] = ker[di, dj]   -> shared tail: 8 images x 14
    output rows packed on the partition axis.
    """
    A = np.zeros((128, 3, BLK), np.float16)
    T8 = np.zeros((128, 3, BPC * TAIL_M), np.float16)
    k16 = ker.astype(np.float16)
    for dj in range(3):
        for di in range(3):
            A[np.arange(BLK) + di, dj, np.arange(BLK)] = k16[di, dj]
            for i in range(BPC):
                T8[
                    i * TAIL_K + np.arange(TAIL_M) + di,
                    dj,
                    i * TAIL_M + np.arange(TAIL_M),
                ] = k16[di, dj]
    return A, T8


def _overlap_in_ap(x, img):
    """DRAM AP reading blocks 0..7 of image `img` as [128p, 8b, 1024c] with
    2-row overlap between consecutive blocks (input row = 126*b + p)."""
    ap = x.ap()[img]
    c = ap.copy()
    c.ap = bass_rust.VecI64Pair([(D, 128), (BLK * D, NBLK), (1, D)])
    return c


def _build(loop_iters=None):
    """Build the per-core Bass program. loop_iters wraps the whole workload
    in a For_i loop (benchmarking variant; kernel() uses loop_iters=None)."""
    nc = bacc.Bacc()
    x = nc.dram_tensor("x", [BPC, D, D], _F16, kind="ExternalInput")
    bandA = nc.dram_tensor("bandA", [128, 3, BLK], _F16, kind="ExternalInput")
    bandT = nc.dram_tensor(
        "bandT", [128, 3, BPC * TAIL_M], _F16, kind="ExternalInput"
    )
    # Block-major output: per image, partition p holds output rows
    # {p, 126+p, ..., 882+p} as 8 contiguous 1022-wide runs. HBM write
    # bandwidth scales with line size (16KB lines ~220 GB/s, 32KB ~370 GB/s),
    # so the 6 middle images are stored in pairs (32KB/partition lines) while
    # images 0 and 7 go as singles to keep pipeline fill/drain short.
    # Host unscrambles. yp[k] = images (2k+1, 2k+2); ys[0], ys[1] = images 0, 7.
    yp = nc.dram_tensor(
        "yp", [BPC // 2 - 1, BLK, 2 * NBLK * O], _F16, kind="ExternalOutput"
    )
    ys = nc.dram_tensor("ys", [2, BLK, NBLK * O], _F16, kind="ExternalOutput")
    # tail rows 1008..1021 of all 8 images, partition-packed (img*14 + q);
    # 1024-wide (2 pad cols) so write lines are 2KB-aligned
    yt = nc.dram_tensor("yt", [BPC * TAIL_M, D], _F16, kind="ExternalOutput")

    with TileContext(nc) as tc:
        with (
            tc.tile_pool(name="bands", bufs=1) as bands,
            tc.tile_pool(name="xin", bufs=6) as xin,
            tc.tile_pool(name="ps", bufs=4, space="PSUM") as ps,
            tc.tile_pool(name="yout", bufs=2) as yout,
        ):
            A = bands.tile([128, 3, BLK], _F16)
            T = bands.tile([128, 3, BPC * TAIL_M], _F16)
            XT = bands.tile([128, D], _F16)
            YT = bands.tile([128, D], _F16)
            nc.vector.memset(YT[:], 0.0)
            nc.sync.dma_start(A[:], bandA[:])
            nc.sync.dma_start(T[:], bandT[:])
            nc.sync.dma_start(XT[:], x[:, D - TAIL_K : D, :])

            def tail_pass():
                # one PSUM tile holds all 8 images' last-14 output rows
                P = ps.tile([128, O], _F32, tag="p")
                for s0, sl in ((0, 512), (512, 510)):
                    for dj in range(3):
                        nc.tensor.matmul(
                            P[: BPC * TAIL_M, s0 : s0 + sl],
                            lhsT=T[:, dj, :],
                            rhs=XT[:, dj + s0 : dj + s0 + sl],
                            start=(dj == 0),
                            stop=(dj == 2),
                        )
                nc.vector.tensor_copy(
                    YT[: BPC * TAIL_M, :O], P[: BPC * TAIL_M, :]
                )
                nc.scalar.dma_start(yt[:, :], YT[: BPC * TAIL_M, :])

            def compute_image(img, Y, slot):
                # load in two half-image DMAs: PE starts on blocks 0-3 while
                # blocks 4-7 are still in flight, hiding DMA-completion
                # semaphore latency and shortening pipeline fill
                X = xin.tile([128, NBLK, D], _F16, tag="x")
                for h in range(2):
                    ap = x.ap()[img]
                    c = ap.copy()
                    c.offset += h * (NBLK // 2) * BLK * D
                    c.ap = bass_rust.VecI64Pair(
                        [(D, 128), (BLK * D, NBLK // 2), (1, D)]
                    )
                    nc.sync.dma_start(
                        X[:, h * (NBLK // 2) : (h + 1) * (NBLK // 2), :], c
                    )
                for b in range(NBLK):
                    P = ps.tile([128, O], _F32, tag="p")
                    for s0, sl in ((0, 512), (512, 510)):
                        for dj in range(3):
                            nc.tensor.matmul(
                                P[:BLK, s0 : s0 + sl],
                                lhsT=A[:, dj, :],
                                rhs=X[:, b, dj + s0 : dj + s0 + sl],
                                start=(dj == 0),
                                stop=(dj == 2),
                            )
                    if b % 2 == 0:
                        nc.scalar.copy(Y[:BLK, slot, b, :], P[:BLK, :])
                    else:
                        nc.vector.tensor_copy(Y[:BLK, slot, b, :], P[:BLK, :])

            def all_images():
                tail_pass()
                # image 0: single store (short pipeline fill)
                Y = yout.tile([128, 1, NBLK, O], _F16, tag="y1")
                compute_image(0, Y, 0)
                nc.scalar.dma_start(
                    ys[0].rearrange("p (b c) -> p b c", b=NBLK), Y[:BLK, 0]
                )
                # images 1..6 in pairs: 32KB/partition store lines
                for k in range(BPC // 2 - 1):
                    Y = yout.tile([128, 2, NBLK, O], _F16, tag="y2")
                    compute_image(2 * k + 1, Y, 0)
                    compute_image(2 * k + 2, Y, 1)
                    nc.scalar.dma_start(
                        yp[k].rearrange("p (i b c) -> p i b c", i=2, b=NBLK),
                        Y[:BLK, :, :, :],
                    )
                # image 7: single store (short pipeline drain)
                Y = yout.tile([128, 1, NBLK, O], _F16, tag="y1")
                compute_image(BPC - 1, Y, 0)
                nc.scalar.dma_start(
                    ys[1].rearrange("p (b c) -> p b c", b=NBLK), Y[:BLK, 0]
                )

            if loop_iters is None:
                all_images()
            else:
                with tc.For_i(0, loop_iters, 1):
                    all_images()
    nc.compile()
    return nc


_CACHE = {}


def _make_runner(nc, donate=True):
    """Wrap a finalized Bass program in a jitted SPMD runner.

    Mirrors run_bass_via_pjrt: operands are (inputs..., zero outputs...,
    partition-id), in exactly the jit parameter order neuronx_cc_hook
    requires.
    """
    import jax
    from jax.sharding import Mesh, PartitionSpec
    from jax.experimental.shard_map import shard_map
    from concourse.bass2jax import (
        _bass_exec_p,
        partition_id_tensor,
        install_neuronx_cc_hook,
    )

    install_neuronx_cc_hook()
    partition_name = nc.partition_id_tensor.name if nc.partition_id_tensor else None

    in_names, out_names, out_avals, zero_outs = [], [], [], []
    for alloc in nc.m.functions[0].allocations:
        if not isinstance(alloc, mybir.MemoryLocationSet):
            continue
        name = alloc.memorylocations[0].name
        if alloc.kind == "ExternalInput":
            if name != partition_name:
                in_names.append(name)
        elif alloc.kind == "ExternalOutput":
            out_names.append(name)
            shape = tuple(alloc.tensor_shape)
            dtype = mybir.dt.np(alloc.dtype)
            out_avals.append(jax.core.ShapedArray(shape, dtype))
            zero_outs.append(np.zeros(shape, dtype))
    n_params = len(in_names)
    n_outs = len(out_avals)
    all_names = in_names + out_names
    if partition_name is not None:
        all_names.append(partition_name)

    def _body(*args):
        outs = _bass_exec_p.bind(
            *args,
            partition_id_tensor(),
            out_avals=tuple(out_avals),
            in_names=tuple(all_names),
            out_names=tuple(out_names),
            lowering_input_output_aliases=(),
            sim_require_finite=True,
            sim_require_nnan=True,
            nc=nc,
        )
        return tuple(outs)

    devices = jax.devices()[:N_CORES]
    mesh = Mesh(np.asarray(devices), ("core",))
    fn = jax.jit(
        shard_map(
            _body,
            mesh=mesh,
            in_specs=(PartitionSpec("core"),) * (n_params + n_outs),
            out_specs=(PartitionSpec("core"),) * n_outs,
            check_rep=False,
        ),
        donate_argnums=(
            tuple(range(n_params, n_params + n_outs)) if donate else ()
        ),
        keep_unused=True,
    )
    return fn, in_names, out_names, zero_outs


def _get_runner(loop_iters=None, donate=True):
    key = ("runner", loop_iters, donate)
    if key not in _CACHE:
        _CACHE[key] = _make_runner(_build(loop_iters), donate=donate)
    return _CACHE[key]


def _concat_inputs(inputs, ker):
    A, T8 = _make_bands(np.asarray(ker, np.float32).reshape(3, 3))
    x16 = (
        np.ascontiguousarray(np.asarray(inputs, np.float32))
        .reshape(B, D, D)
        .astype(np.float16)
    )
    return {
        "x": x16,
        "bandA": np.ascontiguousarray(
            np.broadcast_to(A, (N_CORES,) + A.shape)
        ).reshape(N_CORES * 128, 3, BLK),
        "bandT": np.ascontiguousarray(
            np.broadcast_to(T8, (N_CORES,) + T8.shape)
        ).reshape(N_CORES * 128, 3, BPC * TAIL_M),
    }


def kernel(inputs, kernel):
    import jax

    fn, in_names, out_names, zero_outs = _get_runner()
    concat = _concat_inputs(inputs, kernel)
    zeros = [
        np.zeros((N_CORES * z.shape[0], *z.shape[1:]), z.dtype) for z in zero_outs
    ]
    outs = fn(*[concat[n] for n in in_names], *zeros)
    outs = {n: np.asarray(o) for n, o in zip(out_names, jax.block_until_ready(outs))}
    # block-major pieces: yp [8*3, 126, 2*8*1022] (images 1-6 per core in
    # pairs), ys [8*2, 126, 8*1022] (images 0 and 7), yt [8*112, 1024] tails
    npair = BPC // 2 - 1
    yp32 = outs["yp"].astype(np.float32).reshape(N_CORES, npair, BLK, 2, NBLK, O)
    ys32 = outs["ys"].astype(np.float32).reshape(N_CORES, 2, BLK, NBLK, O)
    y_tail = (
        outs["yt"][:, :O].astype(np.float32).reshape(N_CORES, BPC, TAIL_M, O)
    )
    # Z[core, img, p, b, c] -> rows 126*b + p
    Z = np.empty((N_CORES, BPC, BLK, NBLK, O), np.float32)
    Z[:, 0] = ys32[:, 0]
    Z[:, BPC - 1] = ys32[:, 1]
    Z[:, 1 : BPC - 1] = yp32.transpose(0, 1, 3, 2, 4, 5).reshape(
        N_CORES, 2 * npair, BLK, NBLK, O
    )
    full = np.empty((B, O, O), np.float32)
    full[:, : NBLK * BLK, :] = Z.transpose(0, 1, 3, 2, 4).reshape(
        B, NBLK * BLK, O
    )
    full[:, NBLK * BLK :, :] = y_tail.reshape(B, TAIL_M, O)
    return full.reshape(B, O * O)


# revision 30
# speedup vs baseline: 1.1766x; 1.1766x over previous
"""3x3 valid cross-correlation of 64 1024x1024 f32 images on 8 TRN2 NeuronCores.

Pure data parallel over batch (8 images/core). The conv is memory-bound, so
everything is about HBM traffic and DMA efficiency:

- fp16 I/O: inputs f32->fp16 and outputs fp16->f32 on the host; all device
  DMA moves 2-byte elements (L2 rel err ~3.4e-4, gate is 2e-2).
- Compute: each image is 8 row-blocks (128 input rows -> 126 output rows,
  2-row overlap). TensorEngine does 3 PSUM-accumulated fp16 matmuls per
  512-wide column segment: a banded [128, 126] stationary applies the 3
  vertical taps of kernel column dj; the moving operand is the image block
  column-shifted by dj (free-dim AP offset). The last 14 output rows of ALL
  8 images run in one shared "tail" pass (8x16 input rows packed on 128
  partitions), replacing 8 near-empty per-image tail passes.
- Stores: TRN2 HBM write bandwidth collapses ~5x on small or misaligned
  lines (4088B lines ~82 GB/s vs 32KB lines ~370 GB/s), so outputs are
  written block-major (partition p = rows {p, 126+p, ...} contiguous) and
  unscrambled on the host. The 6 middle images are stored in pairs for
  32KB/partition lines; images 0/7 go as singles to keep fill/drain short.
- Loads ride the SP HWDGE ring (one [128p, 8b, 1024c] overlap-AP DMA per
  image; 2KB row lines — big-line load layouts measured faster standalone
  but stall the PE in the full pipeline), stores the ACT ring.
"""

import numpy as np

import bass_rust
import concourse.bacc as bacc
import concourse.mybir as mybir
from concourse.tile import TileContext

B = 64          # batch
D = 1024        # image side
O = D - 2       # 1022 output side
N_CORES = 8
BPC = B // N_CORES  # images per core
BLK = 126       # output rows per full block
NBLK = 8        # full blocks per image; tail handled separately
TAIL_M = O - NBLK * BLK   # 14 tail output rows per image
TAIL_K = 16     # tail input rows per image (1008..1023)

_F32 = mybir.dt.float32
_F16 = mybir.dt.float16


def _make_bands(ker):
    """Banded stationary matrices from the 3x3 kernel (fp16).

    A[k, dj, m] = ker[k-m, dj]  (k-m in 0..2) -> 126 output rows per block
    T8[i*16+q+di, dj, i*14+q] = ker[di, dj]   -> shared tail: 8 images x 14
    output rows packed on the partition axis.
    """
    A = np.zeros((128, 3, BLK), np.float16)
    T8 = np.zeros((128, 3, BPC * TAIL_M), np.float16)
    k16 = ker.astype(np.float16)
    for dj in range(3):
        for di in range(3):
            A[np.arange(BLK) + di, dj, np.arange(BLK)] = k16[di, dj]
            for i in range(BPC):
                T8[
                    i * TAIL_K + np.arange(TAIL_M) + di,
                    dj,
                    i * TAIL_M + np.arange(TAIL_M),
                ] = k16[di, dj]
    return A, T8


def _overlap_in_ap(x, img):
    """DRAM AP reading blocks 0..7 of image `img` as [128p, 8b, 1024c] with
    2-row overlap between consecutive blocks (input row = 126*b + p)."""
    ap = x.ap()[img]
    c = ap.copy()
    c.ap = bass_rust.VecI64Pair([(D, 128), (BLK * D, NBLK), (1, D)])
    return c


def _build(loop_iters=None):
    """Build the per-core Bass program. loop_iters wraps the whole workload
    in a For_i loop (benchmarking variant; kernel() uses loop_iters=None)."""
    nc = bacc.Bacc()
    x = nc.dram_tensor("x", [BPC, D, D], _F16, kind="ExternalInput")
    bandA = nc.dram_tensor("bandA", [128, 3, BLK], _F16, kind="ExternalInput")
    bandT = nc.dram_tensor(
        "bandT", [128, 3, BPC * TAIL_M], _F16, kind="ExternalInput"
    )
    # Block-major output: per image, partition p holds output rows
    # {p, 126+p, ..., 882+p} as 8 contiguous 1022-wide runs. HBM write
    # bandwidth scales with line size (16KB lines ~220 GB/s, 32KB ~370 GB/s),
    # so the 6 middle images are stored in pairs (32KB/partition lines) while
    # images 0 and 7 go as singles to keep pipeline fill/drain short.
    # Host unscrambles. yp[k] = images (2k+1, 2k+2); ys[0], ys[1] = images 0, 7.
    yp = nc.dram_tensor(
        "yp", [BPC // 2 - 1, BLK, 2 * NBLK * O], _F16, kind="ExternalOutput"
    )
    ys = nc.dram_tensor("ys", [2, BLK, NBLK * O], _F16, kind="ExternalOutput")
    # tail rows 1008..1021 of all 8 images, partition-packed (img*14 + q);
    # 1024-wide (2 pad cols) so write lines are 2KB-aligned
    yt = nc.dram_tensor("yt", [BPC * TAIL_M, D], _F16, kind="ExternalOutput")

    with TileContext(nc) as tc:
        with (
            tc.tile_pool(name="bands", bufs=1) as bands,
            tc.tile_pool(name="xin", bufs=4) as xin,
            tc.tile_pool(name="ps", bufs=4, space="PSUM") as ps,
            tc.tile_pool(name="yout", bufs=2) as yout,
        ):
            A = bands.tile([128, 3, BLK], _F16)
            T = bands.tile([128, 3, BPC * TAIL_M], _F16)
            XT = bands.tile([128, D], _F16)
            YT = bands.tile([128, D], _F16)
            nc.vector.memset(YT[:], 0.0)
            nc.sync.dma_start(A[:], bandA[:])
            nc.sync.dma_start(T[:], bandT[:])
            nc.sync.dma_start(XT[:], x[:, D - TAIL_K : D, :])

            def tail_pass():
                # one PSUM tile holds all 8 images' last-14 output rows
                P = ps.tile([128, O], _F32, tag="p")
                for s0, sl in ((0, 512), (512, 510)):
                    for dj in range(3):
                        nc.tensor.matmul(
                            P[: BPC * TAIL_M, s0 : s0 + sl],
                            lhsT=T[:, dj, :],
                            rhs=XT[:, dj + s0 : dj + s0 + sl],
                            start=(dj == 0),
                            stop=(dj == 2),
                        )
                nc.vector.tensor_copy(
                    YT[: BPC * TAIL_M, :O], P[: BPC * TAIL_M, :]
                )
                nc.scalar.dma_start(yt[:, :], YT[: BPC * TAIL_M, :])

            def compute_image(img, Y, slot):
                X = xin.tile([128, NBLK, D], _F16, tag="x")
                nc.sync.dma_start(X[:], _overlap_in_ap(x, img))
                for b in range(NBLK):
                    P = ps.tile([128, O], _F32, tag="p")
                    for s0, sl in ((0, 512), (512, 510)):
                        for dj in range(3):
                            nc.tensor.matmul(
                                P[:BLK, s0 : s0 + sl],
                                lhsT=A[:, dj, :],
                                rhs=X[:, b, dj + s0 : dj + s0 + sl],
                                start=(dj == 0),
                                stop=(dj == 2),
                            )
                    if b % 2 == 0:
                        nc.scalar.copy(Y[:BLK, slot, b, :], P[:BLK, :])
                    else:
                        nc.vector.tensor_copy(Y[:BLK, slot, b, :], P[:BLK, :])

            def all_images():
                tail_pass()
                # image 0: single store (short pipeline fill)
                Y = yout.tile([128, 1, NBLK, O], _F16, tag="y1")
                compute_image(0, Y, 0)
                nc.scalar.dma_start(
                    ys[0].rearrange("p (b c) -> p b c", b=NBLK), Y[:BLK, 0]
                )
                # images 1..6 in pairs: 32KB/partition store lines
                for k in range(BPC // 2 - 1):
                    Y = yout.tile([128, 2, NBLK, O], _F16, tag="y2")
                    compute_image(2 * k + 1, Y, 0)
                    compute_image(2 * k + 2, Y, 1)
                    nc.scalar.dma_start(
                        yp[k].rearrange("p (i b c) -> p i b c", i=2, b=NBLK),
                        Y[:BLK, :, :, :],
                    )
                # image 7: single store (short pipeline drain)
                Y = yout.tile([128, 1, NBLK, O], _F16, tag="y1")
                compute_image(BPC - 1, Y, 0)
                nc.scalar.dma_start(
                    ys[1].rearrange("p (b c) -> p b c", b=NBLK), Y[:BLK, 0]
                )

            if loop_iters is None:
                all_images()
            else:
                with tc.For_i(0, loop_iters, 1):
                    all_images()
    nc.compile()
    return nc


_CACHE = {}


def _make_runner(nc, donate=True):
    """Wrap a finalized Bass program in a jitted SPMD runner.

    Mirrors run_bass_via_pjrt: operands are (inputs..., zero outputs...,
    partition-id), in exactly the jit parameter order neuronx_cc_hook
    requires.
    """
    import jax
    from jax.sharding import Mesh, PartitionSpec
    from jax.experimental.shard_map import shard_map
    from concourse.bass2jax import (
        _bass_exec_p,
        partition_id_tensor,
        install_neuronx_cc_hook,
    )

    install_neuronx_cc_hook()
    partition_name = nc.partition_id_tensor.name if nc.partition_id_tensor else None

    in_names, out_names, out_avals, zero_outs = [], [], [], []
    for alloc in nc.m.functions[0].allocations:
        if not isinstance(alloc, mybir.MemoryLocationSet):
            continue
        name = alloc.memorylocations[0].name
        if alloc.kind == "ExternalInput":
            if name != partition_name:
                in_names.append(name)
        elif alloc.kind == "ExternalOutput":
            out_names.append(name)
            shape = tuple(alloc.tensor_shape)
            dtype = mybir.dt.np(alloc.dtype)
            out_avals.append(jax.core.ShapedArray(shape, dtype))
            zero_outs.append(np.zeros(shape, dtype))
    n_params = len(in_names)
    n_outs = len(out_avals)
    all_names = in_names + out_names
    if partition_name is not None:
        all_names.append(partition_name)

    def _body(*args):
        outs = _bass_exec_p.bind(
            *args,
            partition_id_tensor(),
            out_avals=tuple(out_avals),
            in_names=tuple(all_names),
            out_names=tuple(out_names),
            lowering_input_output_aliases=(),
            sim_require_finite=True,
            sim_require_nnan=True,
            nc=nc,
        )
        return tuple(outs)

    devices = jax.devices()[:N_CORES]
    mesh = Mesh(np.asarray(devices), ("core",))
    fn = jax.jit(
        shard_map(
            _body,
            mesh=mesh,
            in_specs=(PartitionSpec("core"),) * (n_params + n_outs),
            out_specs=(PartitionSpec("core"),) * n_outs,
            check_rep=False,
        ),
        donate_argnums=(
            tuple(range(n_params, n_params + n_outs)) if donate else ()
        ),
        keep_unused=True,
    )
    return fn, in_names, out_names, zero_outs


def _get_runner(loop_iters=None, donate=True):
    key = ("runner", loop_iters, donate)
    if key not in _CACHE:
        _CACHE[key] = _make_runner(_build(loop_iters), donate=donate)
    return _CACHE[key]


def _concat_inputs(inputs, ker):
    A, T8 = _make_bands(np.asarray(ker, np.float32).reshape(3, 3))
    x16 = (
        np.ascontiguousarray(np.asarray(inputs, np.float32))
        .reshape(B, D, D)
        .astype(np.float16)
    )
    return {
        "x": x16,
        "bandA": np.ascontiguousarray(
            np.broadcast_to(A, (N_CORES,) + A.shape)
        ).reshape(N_CORES * 128, 3, BLK),
        "bandT": np.ascontiguousarray(
            np.broadcast_to(T8, (N_CORES,) + T8.shape)
        ).reshape(N_CORES * 128, 3, BPC * TAIL_M),
    }


def kernel(inputs, kernel):
    import jax

    fn, in_names, out_names, zero_outs = _get_runner()
    concat = _concat_inputs(inputs, kernel)
    zeros = [
        np.zeros((N_CORES * z.shape[0], *z.shape[1:]), z.dtype) for z in zero_outs
    ]
    outs = fn(*[concat[n] for n in in_names], *zeros)
    outs = {n: np.asarray(o) for n, o in zip(out_names, jax.block_until_ready(outs))}
    # block-major pieces: yp [8*3, 126, 2*8*1022] (images 1-6 per core in
    # pairs), ys [8*2, 126, 8*1022] (images 0 and 7), yt [8*112, 1024] tails
    npair = BPC // 2 - 1
    yp32 = outs["yp"].astype(np.float32).reshape(N_CORES, npair, BLK, 2, NBLK, O)
    ys32 = outs["ys"].astype(np.float32).reshape(N_CORES, 2, BLK, NBLK, O)
    y_tail = (
        outs["yt"][:, :O].astype(np.float32).reshape(N_CORES, BPC, TAIL_M, O)
    )
    # Z[core, img, p, b, c] -> rows 126*b + p
    Z = np.empty((N_CORES, BPC, BLK, NBLK, O), np.float32)
    Z[:, 0] = ys32[:, 0]
    Z[:, BPC - 1] = ys32[:, 1]
    Z[:, 1 : BPC - 1] = yp32.transpose(0, 1, 3, 2, 4, 5).reshape(
        N_CORES, 2 * npair, BLK, NBLK, O
    )
    full = np.empty((B, O, O), np.float32)
    full[:, : NBLK * BLK, :] = Z.transpose(0, 1, 3, 2, 4).reshape(
        B, NBLK * BLK, O
    )
    full[:, NBLK * BLK :, :] = y_tail.reshape(B, TAIL_M, O)
    return full.reshape(B, O * O)
